# revision 2
# baseline (speedup 1.0000x reference)
"""Trainium2 Bass kernel for nn_Bootstrap_Proposal (time != 0 branch).

Math (L1=L2=M1=M2=1, DT=0.01), per particle with state
[tq1, tq2, th1, th2, v1, v2]:

    ss  = sin(th2/2)^2        (cos via half-angle; ACT Sin domain is [-pi,pi])
    g   = d01 = 5/6 - ss
    det = 4/9 - (1/2 - ss)^2
    a1  = ( tq1/3 - g*tq2 ) / det
    a2  = ( (2g+1)*tq2 - g*tq1 ) / det
    out = [tq1, tq2, th1 + DT*v1, th2 + DT*v2, v1 + DT*a1, v2 + DT*a2]

Sharding: pure data parallel over the batch axis, 8 cores. The rel-err gate
is 2e-2, so all device I/O is bf16 (costs ~6e-3): per core the input shard
[16, 16384, 6] becomes a [128, 12288] bf16 block (partition p owns 2048
consecutive particles), host pre-transposed to per-tile channel-planar
layout [6, w] per row so every engine op is unit-stride. The device only
writes the 4 computed channels ([4, w] planes: th1', th2', v1', v2'); the
torque channels pass through on the host from the ORIGINAL f32 input
(exact). HBM traffic per core: 3.15 MB in + 2.10 MB out (vs 12.6 MB for
the f32 in-place variant).

Engine split per tile: ACT does the 4-op transcendental chain (Sin, Square,
Square, Copy; g is folded into DVE scalar_tensor_tensor constants), DVE the
rational chain + the two angle updates, Pool the two a*rr muls and the two
velocity updates.
"""

import numpy as np
from contextlib import ExitStack

from concourse import bacc, tile, mybir
from concourse.alu_op_type import AluOpType
from concourse.bass_utils import run_bass_kernel_spmd

N_CORES = 8
B, P, C = 128, 16384, 6
ROWS = 128
PART = (B // N_CORES) * P // ROWS      # 2048 particles per partition per core
NI = C * PART                          # 12288 bf16 in per row
NO = 4 * PART                          # 8192 bf16 out per row
DT = 0.01
F32 = mybir.dt.float32
BF16 = mybir.dt.bfloat16


def _build_nc(n_t=4, io_bufs=3, tmp_bufs=2, reps=1, body="full",
              store_engine="sync", o23_engine="vector", mul_engine="gpsimd",
              add45_engine="gpsimd"):
    # Bacc (not raw Bass): its compile() pass pipeline splits multi-sem waits
    # (walrus allows one sync wait per instruction) and allocates registers.
    nc = bacc.Bacc(
        "TRN2",
        target_bir_lowering=False,
        debug=False,
        num_devices=N_CORES,
    )
    assert PART % n_t == 0
    w = PART // n_t
    x = nc.dram_tensor("x", [ROWS, NI], BF16, kind="ExternalInput").ap()
    y = nc.dram_tensor("y", [ROWS, NO], BF16, kind="ExternalOutput").ap()

    Sin = mybir.ActivationFunctionType.Sin
    Square = mybir.ActivationFunctionType.Square
    Copy = mybir.ActivationFunctionType.Copy
    mult, add, sub = AluOpType.mult, AluOpType.add, AluOpType.subtract

    # activation() lowers non-Copy float biases through the const-AP table;
    # only 0.0/1.0 are pre-registered, so add the 0.5 we use for Square.
    cb = nc.alloc_sbuf_tensor("const-f32-half", [128, 1], F32)
    nc.gpsimd.memset(cb.ap(), 0.5)
    nc.const_aps.aps[(F32, 0.5)] = cb.ap()
    nc.all_engine_barrier()

    store_eng = nc.sync if store_engine == "sync" else nc.scalar
    o23 = nc.vector if o23_engine == "vector" else nc.gpsimd
    mle = nc.gpsimd if mul_engine == "gpsimd" else nc.vector
    a45 = nc.gpsimd if add45_engine == "gpsimd" else nc.vector

    with tile.TileContext(nc) as tc, ExitStack() as ctx:
        io = ctx.enter_context(tc.tile_pool(name="io", bufs=io_bufs))
        tmp = ctx.enter_context(tc.tile_pool(name="tmp", bufs=tmp_bufs))

        loop = tc.For_i(0, reps, 1) if reps > 1 else None
        if loop is not None:
            ctx.enter_context(loop)

        for j in range(n_t):
            t = io.tile([ROWS, C * w], BF16, tag="t")
            nc.sync.dma_start(out=t, in_=x[:, j * C * w:(j + 1) * C * w])

            if body == "dma":
                o = io.tile([ROWS, 4 * w], BF16, tag="o")
                store_eng.dma_start(out=y[:, j * 4 * w:(j + 1) * 4 * w],
                                    in_=t[:, : 4 * w])
                continue

            tq1 = t[:, 0 * w:1 * w]
            tq2 = t[:, 1 * w:2 * w]
            th1 = t[:, 2 * w:3 * w]
            th2 = t[:, 3 * w:4 * w]
            v1 = t[:, 4 * w:5 * w]
            v2 = t[:, 5 * w:6 * w]

            o = io.tile([ROWS, 4 * w], BF16, tag="o")
            o2 = o[:, 0 * w:1 * w]
            o3 = o[:, 1 * w:2 * w]
            o4 = o[:, 2 * w:3 * w]
            o5 = o[:, 3 * w:4 * w]

            s = tmp.tile([ROWS, w], F32, tag="s")
            ss = tmp.tile([ROWS, w], F32, tag="ss")
            dd = tmp.tile([ROWS, w], F32, tag="dd")
            det = tmp.tile([ROWS, w], F32, tag="det")
            rr = tmp.tile([ROWS, w], F32, tag="rr")
            tn = tmp.tile([ROWS, w], F32, tag="tn")
            un = tmp.tile([ROWS, w], F32, tag="un")
            n1 = tmp.tile([ROWS, w], F32, tag="n1")
            n2 = tmp.tile([ROWS, w], F32, tag="n2")
            m1 = tmp.tile([ROWS, w], F32, tag="m1")
            m2 = tmp.tile([ROWS, w], F32, tag="m2")

            # ---- ACT: transcendental chain (critical path to rr) ----
            nc.scalar.activation(s, th2, Sin, scale=0.5)                 # sin(th2/2)
            nc.scalar.activation(ss, s, Square)                          # ss
            nc.scalar.activation(dd, ss, Square, bias=0.5, scale=-1.0)   # (1/2-ss)^2
            # det*100 so that 1/det100 = 0.01/det folds DT into the reciprocal
            nc.scalar.activation(det, dd, Copy, bias=400.0 / 9.0, scale=-100.0)

            # ---- DVE: rational chain (g = 5/6 - ss folded into the stt consts) ----
            nc.vector.reciprocal_approx_fast(rr, det)                    # 0.01/det
            nc.vector.scalar_tensor_tensor(tn, ss, 5.0 / 6.0, tq2, sub, mult)  # -g*tq2
            nc.vector.scalar_tensor_tensor(un, ss, 5.0 / 6.0, tq1, sub, mult)  # -g*tq1
            nc.vector.scalar_tensor_tensor(n1, tq1, 1.0 / 3.0, tn, mult, add)  # tq1/3 - g*tq2
            nc.vector.scalar_tensor_tensor(n2, tn, -2.0, tq2, mult, add)       # (2g+1)*tq2
            nc.vector.tensor_tensor(n2, n2, un, add)                           # ... - g*tq1

            # ---- angle updates: o2 = th1 + DT*v1, o3 = th2 + DT*v2 ----
            o23.scalar_tensor_tensor(o2, v1, DT, th1, mult, add)
            o23.scalar_tensor_tensor(o3, v2, DT, th2, mult, add)

            # ---- velocity updates: o4 = v1 + n1*rr, o5 = v2 + n2*rr ----
            mle.tensor_tensor(m1, n1, rr, mult)                          # DT*a1
            mle.tensor_tensor(m2, n2, rr, mult)                          # DT*a2
            a45.tensor_tensor(o4, v1, m1, add)
            a45.tensor_tensor(o5, v2, m2, add)

            store_eng.dma_start(out=y[:, j * 4 * w:(j + 1) * 4 * w], in_=o)
    nc.finalize()
    return nc


_nc_cache = None

BEST = dict(
    n_t=4,
    io_bufs=3,
    tmp_bufs=2,
)


def _get_nc():
    global _nc_cache
    if _nc_cache is None:
        _nc_cache = _build_nc(**BEST)
    return _nc_cache


def _shard_in(prev, n_t):
    """[B, P, 6] f32 -> [8, 128, NI] bf16, per-tile channel-planar."""
    import ml_dtypes

    w = PART // n_t
    planar = (
        prev.reshape(N_CORES, ROWS, n_t, w, C)
        .transpose(0, 1, 2, 4, 3)
        .reshape(N_CORES, ROWS, NI)
    )
    return np.ascontiguousarray(planar.astype(ml_dtypes.bfloat16))


def _unshard_out(y, prev, n_t):
    """[8, 128, NO] bf16 + original f32 input -> [B, P, 6] f32."""
    w = PART // n_t
    ch = (
        y.astype(np.float32)
        .reshape(N_CORES, ROWS, n_t, 4, w)
        .transpose(0, 1, 2, 4, 3)
        .reshape(B, P, 4)
    )
    out = np.empty((B, P, C), np.float32)
    out[..., 0:2] = prev[..., 0:2]
    out[..., 2:6] = ch
    return out


def run(prev_latents, trace=False, **trace_kwargs):
    prev = np.ascontiguousarray(np.asarray(prev_latents, dtype=np.float32))
    assert prev.shape == (B, P, C), prev.shape
    n_t = BEST["n_t"]
    shards = _shard_in(prev, n_t)
    in_maps = [{"x": shards[i]} for i in range(N_CORES)]
    res = run_bass_kernel_spmd(
        _get_nc(), in_maps, list(range(N_CORES)), trace=trace, **trace_kwargs
    )
    y = np.stack([np.asarray(res.results[i]["y"]) for i in range(N_CORES)])
    return _unshard_out(y, prev, n_t), res


def kernel(**inputs):
    out, _ = run(inputs["prev_latents"])
    return out


def make_timed_runner():
    """Build a reusable jitted SPMD callable mirroring run_bass_via_pjrt's
    multi-core branch, for steady-state HW timing."""
    import jax
    from jax.sharding import Mesh, NamedSharding, PartitionSpec
    from jax.experimental.shard_map import shard_map
    from concourse import bass2jax

    nc = _get_nc()
    bass2jax.install_neuronx_cc_hook()
    partition_name = nc.partition_id_tensor.name if nc.partition_id_tensor else None

    in_names, out_names, out_avals, zero_outs = [], [], [], []
    for alloc in nc.m.functions[0].allocations:
        if not isinstance(alloc, mybir.MemoryLocationSet):
            continue
        name = alloc.memorylocations[0].name
        if alloc.kind == "ExternalInput":
            if name != partition_name:
                in_names.append(name)
        elif alloc.kind == "ExternalOutput":
            out_names.append(name)
            shape = tuple(alloc.tensor_shape)
            dtype = mybir.dt.np(alloc.dtype)
            out_avals.append(jax.core.ShapedArray(shape, dtype))
            zero_outs.append(np.zeros(shape, dtype))
    n_params, n_outs = len(in_names), len(out_avals)
    in_names.extend(out_names)
    if partition_name is not None:
        in_names.append(partition_name)
    donate = tuple(range(n_params, n_params + n_outs))

    def _body(*args):
        operands = list(args)
        if partition_name is not None:
            operands.append(bass2jax.partition_id_tensor())
        outs = bass2jax._bass_exec_p.bind(
            *operands,
            out_avals=tuple(out_avals),
            in_names=tuple(in_names),
            out_names=tuple(out_names),
            lowering_input_output_aliases=(),
            sim_require_finite=True,
            sim_require_nnan=True,
            nc=nc,
        )
        return tuple(outs)

    devices = jax.devices()[:N_CORES]
    mesh = Mesh(np.asarray(devices), ("core",))
    spec = PartitionSpec("core")
    step = jax.jit(
        shard_map(
            _body,
            mesh=mesh,
            in_specs=(spec,) * (n_params + n_outs),
            out_specs=(spec,) * n_outs,
            check_rep=False,
        ),
        donate_argnums=donate,
        keep_unused=True,
    )

    def place(arr):
        return jax.device_put(arr, NamedSharding(mesh, spec))

    concat_zeros = [
        np.zeros((N_CORES * z.shape[0], *z.shape[1:]), z.dtype) for z in zero_outs
    ]
    return step, place, concat_zeros


# revision 6
# speedup vs baseline: 1.0054x; 1.0054x over previous
"""Trainium2 Bass kernel for nn_Bootstrap_Proposal (time != 0 branch).

Math (L1=L2=M1=M2=1, DT=0.01), per particle with state
[tq1, tq2, th1, th2, v1, v2]:

    ss  = sin(th2/2)^2        (cos via half-angle; ACT Sin domain is [-pi,pi])
    g   = d01 = 5/6 - ss
    det = 4/9 - (1/2 - ss)^2
    a1  = ( tq1/3 - g*tq2 ) / det
    a2  = ( (2g+1)*tq2 - g*tq1 ) / det
    out = [tq1, tq2, th1 + DT*v1, th2 + DT*v2, v1 + DT*a1, v2 + DT*a2]

Sharding: pure data parallel over the batch axis, 8 cores. The rel-err gate
is 2e-2, so the whole kernel runs in bf16 (costs ~6e-3): per core the input
shard [16, 16384, 6] becomes a [128, 12288] bf16 block (partition p owns
2048 consecutive particles), host pre-transposed to per-tile channel-planar
layout [6, w] per row so every engine op is unit-stride and pairs of
channels can be processed as one [2w] op. The device only writes the 4
computed channels ([4, w] planes: th1', th2', v1', v2'); the torque
channels pass through on the host from the ORIGINAL f32 input (exact).
HBM traffic per core: 3.15 MB in + 2.10 MB out.

Cost-model facts this schedule is built around (per [128, w] op, w=2048):
  - DVE tensor_tensor gets a 2x speed mode iff ALL tensor operands are
    2-byte packed (1127ns); scalar_tensor_tensor never does (2194ns).
  - Pool tensor_tensor runs at 0.42 sw-efficiency (4158ns) - use sparingly.
  - ACT activation is 1892ns regardless of dtype; Sin and Reciprocal live
    in different table sets (1283ns LoadActFuncSet each way), Copy/Square
    are in both.
  - 0.01/det is computed on ACT via the Reciprocal table (emitted raw: the
    bass accuracy guard is irrelevant here since accel errors are scaled
    by DT), which keeps DVE free of the f32-only reciprocal and makes the
    whole DVE chain bf16/2x.

Engine split per tile: ACT 5 ops (Sin, Square ss, Square dd, Copy g,
Reciprocal rrD), DVE 5 tt @2x + 2 stt + o23 stt [2w], Pool o45 tt [2w].
"""

import numpy as np
from contextlib import ExitStack

from concourse import bacc, tile, mybir
from concourse.alu_op_type import AluOpType
from concourse.bass_utils import run_bass_kernel_spmd

N_CORES = 8
B, P, C = 128, 16384, 6
ROWS = 128
PART = (B // N_CORES) * P // ROWS      # 2048 particles per partition per core
NI = C * PART                          # 12288 bf16 in per row
NO = 4 * PART                          # 8192 bf16 out per row
DT = 0.01
F32 = mybir.dt.float32
BF16 = mybir.dt.bfloat16


def _act_raw(nc, out, in_, func, bias=0.0, scale=1.0):
    """activation() minus the Reciprocal accuracy guard (see module doc)."""
    eng = nc.scalar
    ins = [eng.lower_ap(in_)]
    for arg in (bias, scale, 0.0):
        ins.append(mybir.ImmediateValue(dtype=mybir.dt.float32, value=arg))
    return eng.add_instruction(
        mybir.InstActivation(
            name=eng.bass.get_next_instruction_name(),
            func=func,
            ins=ins,
            outs=[eng.lower_ap(out)],
        )
    )


def _build_nc(io_bufs=3, tmp_bufs=2, reps=1, body="full",
              load_engine="scalar", store_engine="sync", recip_engine="act",
              o45_engine="gpsimd", o23_engine="vector"):
    # Bacc (not raw Bass): its compile() pass pipeline splits multi-sem waits
    # (walrus allows one sync wait per instruction) and allocates registers.
    nc = bacc.Bacc(
        "TRN2",
        target_bir_lowering=False,
        debug=False,
        num_devices=N_CORES,
    )
    w = PART  # one [128, 6w] tile per iteration
    x = nc.dram_tensor("x", [ROWS, NI], BF16, kind="ExternalInput").ap()
    y = nc.dram_tensor("y", [ROWS, NO], BF16, kind="ExternalOutput").ap()

    Sin = mybir.ActivationFunctionType.Sin
    Square = mybir.ActivationFunctionType.Square
    Copy = mybir.ActivationFunctionType.Copy
    Recip = mybir.ActivationFunctionType.Reciprocal
    mult, add, sub = AluOpType.mult, AluOpType.add, AluOpType.subtract

    # activation() lowers non-Copy float biases through the const-AP table;
    # only 0.0/1.0 are pre-registered, so add the 0.5 we use for Square.
    cb = nc.alloc_sbuf_tensor("const-f32-half", [128, 1], F32)
    nc.gpsimd.memset(cb.ap(), 0.5)
    nc.const_aps.aps[(F32, 0.5)] = cb.ap()
    nc.all_engine_barrier()

    # Only SP/Activation queues can issue HWDGE DMAs. Loads go on the ACT
    # queue (they wait only on buffer-free, which resolves early, so they
    # never stall ACT work); stores sit alone on SP so their sem-wait at
    # the queue head cannot block the next iteration's load.
    engs = {"sync": nc.sync, "scalar": nc.scalar, "gpsimd": nc.gpsimd}
    load_eng = engs[load_engine]
    store_eng = engs[store_engine]
    o45e = nc.gpsimd if o45_engine == "gpsimd" else nc.vector
    o23e = nc.vector if o23_engine == "vector" else nc.gpsimd

    with tile.TileContext(nc) as tc, ExitStack() as ctx:
        io = ctx.enter_context(tc.tile_pool(name="io", bufs=io_bufs))
        tmp = ctx.enter_context(tc.tile_pool(name="tmp", bufs=tmp_bufs))

        loop = tc.For_i(0, reps, 1) if reps > 1 else None
        if loop is not None:
            ctx.enter_context(loop)

        t = io.tile([ROWS, C * w], BF16, tag="t")
        load_eng.dma_start(out=t, in_=x)

        tq1 = t[:, 0 * w:1 * w]
        tq2 = t[:, 1 * w:2 * w]
        th12 = t[:, 2 * w:4 * w]
        th2 = t[:, 3 * w:4 * w]
        v12 = t[:, 4 * w:6 * w]
        # outputs land in-place in planes 2..5 of t -> one contiguous store
        o23 = th12
        o45 = v12

        if body == "dma":
            store_eng.dma_start(out=y, in_=t[:, 2 * w:6 * w])
        else:
            s = tmp.tile([ROWS, w], BF16, tag="s")
            ss = tmp.tile([ROWS, w], BF16, tag="ss")
            dd = tmp.tile([ROWS, w], BF16, tag="dd")
            g = tmp.tile([ROWS, w], BF16, tag="g")
            gm1 = tmp.tile([ROWS, w], BF16, tag="gm1")
            gm2 = tmp.tile([ROWS, w], BF16, tag="gm2")
            n1 = tmp.tile([ROWS, w], BF16, tag="n1")
            n2 = tmp.tile([ROWS, w], BF16, tag="n2")
            m12 = tmp.tile([ROWS, 2 * w], BF16, tag="m12")
            m1 = m12[:, 0:w]
            m2 = m12[:, w:2 * w]
            rr = s  # s is dead after ss; same-engine reuse is hazard-free

            # ---- DVE: o23 first, it depends only on the load ----
            o23e.scalar_tensor_tensor(o23, v12, DT, th12, mult, add)

            # ---- ACT: transcendental chain (critical path to rr) ----
            nc.scalar.activation(s, th2, Sin, scale=0.5)                 # sin(th2/2)
            nc.scalar.activation(ss, s, Square)                          # ss
            nc.scalar.activation(g, ss, Copy, bias=5.0 / 6.0, scale=-1.0)
            nc.scalar.activation(dd, ss, Square, bias=0.5, scale=-1.0)   # (1/2-ss)^2
            if recip_engine == "act":
                # rr = 1/(-100*dd + 400/9) = 0.01/det  (DT folded in).
                # Sin and Reciprocal are in different ACT table sets: this
                # costs 2 LoadActFuncSet (2x 1283ns) per iteration.
                _act_raw(nc, rr, dd, Recip, bias=400.0 / 9.0, scale=-100.0)
            else:
                det = tmp.tile([ROWS, w], F32, tag="det")
                rf = tmp.tile([ROWS, w], F32, tag="rf")
                nc.scalar.activation(det, dd, Copy, bias=400.0 / 9.0, scale=-100.0)
                nc.vector.reciprocal_approx_fast(rf, det)
                nc.scalar.activation(rr, rf, Copy)                       # -> bf16

            # ---- DVE: rational chain (all-bf16 keeps tt ops in 2x mode) ----
            nc.vector.tensor_tensor(gm1, g, tq1, mult)                   # g*tq1
            nc.vector.tensor_tensor(gm2, g, tq2, mult)                   # g*tq2
            nc.vector.scalar_tensor_tensor(n1, tq1, 1.0 / 3.0, gm2, mult, sub)
            nc.vector.scalar_tensor_tensor(n2, gm2, 2.0, tq2, mult, add)
            nc.vector.tensor_tensor(n2, n2, gm1, sub)                    # (2g+1)tq2 - g*tq1
            nc.vector.tensor_tensor(m1, n1, rr, mult)                    # DT*a1
            nc.vector.tensor_tensor(m2, n2, rr, mult)                    # DT*a2

            # ---- velocity updates: o45 = v12 + m12  (one [2w] op) ----
            o45e.tensor_tensor(o45, v12, m12, add)

            store_eng.dma_start(out=y, in_=t[:, 2 * w:6 * w])
    nc.finalize()
    return nc


_nc_cache = None

BEST = dict(
    io_bufs=3,
    tmp_bufs=2,
)


def _get_nc():
    global _nc_cache
    if _nc_cache is None:
        _nc_cache = _build_nc(**BEST)
    return _nc_cache


def _shard_in(prev):
    """[B, P, 6] f32 -> [8, 128, NI] bf16, channel-planar per row."""
    import ml_dtypes

    planar = (
        prev.reshape(N_CORES, ROWS, PART, C)
        .transpose(0, 1, 3, 2)
        .reshape(N_CORES, ROWS, NI)
    )
    return np.ascontiguousarray(planar.astype(ml_dtypes.bfloat16))


def _unshard_out(y, prev):
    """[8, 128, NO] bf16 + original f32 input -> [B, P, 6] f32."""
    ch = (
        y.astype(np.float32)
        .reshape(N_CORES, ROWS, 4, PART)
        .transpose(0, 1, 3, 2)
        .reshape(B, P, 4)
    )
    out = np.empty((B, P, C), np.float32)
    out[..., 0:2] = prev[..., 0:2]
    out[..., 2:6] = ch
    return out


def run(prev_latents, trace=False, **trace_kwargs):
    prev = np.ascontiguousarray(np.asarray(prev_latents, dtype=np.float32))
    assert prev.shape == (B, P, C), prev.shape
    shards = _shard_in(prev)
    in_maps = [{"x": shards[i]} for i in range(N_CORES)]
    res = run_bass_kernel_spmd(
        _get_nc(), in_maps, list(range(N_CORES)), trace=trace, **trace_kwargs
    )
    y = np.stack([np.asarray(res.results[i]["y"]) for i in range(N_CORES)])
    return _unshard_out(y, prev), res


def kernel(**inputs):
    out, _ = run(inputs["prev_latents"])
    return out


def make_timed_runner():
    """Build a reusable jitted SPMD callable mirroring run_bass_via_pjrt's
    multi-core branch, for steady-state HW timing."""
    import jax
    from jax.sharding import Mesh, NamedSharding, PartitionSpec
    from jax.experimental.shard_map import shard_map
    from concourse import bass2jax

    nc = _get_nc()
    bass2jax.install_neuronx_cc_hook()
    partition_name = nc.partition_id_tensor.name if nc.partition_id_tensor else None

    in_names, out_names, out_avals, zero_outs = [], [], [], []
    for alloc in nc.m.functions[0].allocations:
        if not isinstance(alloc, mybir.MemoryLocationSet):
            continue
        name = alloc.memorylocations[0].name
        if alloc.kind == "ExternalInput":
            if name != partition_name:
                in_names.append(name)
        elif alloc.kind == "ExternalOutput":
            out_names.append(name)
            shape = tuple(alloc.tensor_shape)
            dtype = mybir.dt.np(alloc.dtype)
            out_avals.append(jax.core.ShapedArray(shape, dtype))
            zero_outs.append(np.zeros(shape, dtype))
    n_params, n_outs = len(in_names), len(out_avals)
    in_names.extend(out_names)
    if partition_name is not None:
        in_names.append(partition_name)
    donate = tuple(range(n_params, n_params + n_outs))

    def _body(*args):
        operands = list(args)
        if partition_name is not None:
            operands.append(bass2jax.partition_id_tensor())
        outs = bass2jax._bass_exec_p.bind(
            *operands,
            out_avals=tuple(out_avals),
            in_names=tuple(in_names),
            out_names=tuple(out_names),
            lowering_input_output_aliases=(),
            sim_require_finite=True,
            sim_require_nnan=True,
            nc=nc,
        )
        return tuple(outs)

    devices = jax.devices()[:N_CORES]
    mesh = Mesh(np.asarray(devices), ("core",))
    spec = PartitionSpec("core")
    step = jax.jit(
        shard_map(
            _body,
            mesh=mesh,
            in_specs=(spec,) * (n_params + n_outs),
            out_specs=(spec,) * n_outs,
            check_rep=False,
        ),
        donate_argnums=donate,
        keep_unused=True,
    )

    def place(arr):
        return jax.device_put(arr, NamedSharding(mesh, spec))

    concat_zeros = [
        np.zeros((N_CORES * z.shape[0], *z.shape[1:]), z.dtype) for z in zero_outs
    ]
    return step, place, concat_zeros


# revision 11
# speedup vs baseline: 1.2168x; 1.2103x over previous
"""Trainium2 Bass kernel for nn_Bootstrap_Proposal (time != 0 branch).

Math (L1=L2=M1=M2=1, DT=0.01), per particle with state
[tq1, tq2, th1, th2, v1, v2]:

    ss  = sin(th2/2)^2        (cos via half-angle; ACT Sin domain is [-pi,pi])
    g   = d01 = 5/6 - ss
    det = 4/9 - (1/2 - ss)^2
    a1  = ( tq1/3 - g*tq2 ) / det
    a2  = ( (2g+1)*tq2 - g*tq1 ) / det
    out = [tq1, tq2, th1 + DT*v1, th2 + DT*v2, v1 + DT*a1, v2 + DT*a2]

Sharding: pure data parallel over the batch axis, 8 cores. The rel-err gate
is 2e-2, so the whole kernel runs in bf16 (costs ~6e-3): per core the input
shard [16, 16384, 6] becomes a [128, 12288] bf16 block (partition p owns
2048 consecutive particles), host pre-transposed to per-tile channel-planar
layout [6, w] per row so every engine op is unit-stride and pairs of
channels can be processed as one [2w] op. The device only writes the 4
computed channels ([4, w] planes: th1', th2', v1', v2'); the torque
channels pass through on the host from the ORIGINAL f32 input (exact).
HBM traffic per core: 3.15 MB in + 2.10 MB out.

Cost-model facts this schedule is built around (per [128, w] op, w=2048):
  - DVE tensor_tensor gets a 2x speed mode iff ALL tensor operands are
    2-byte packed (1127ns); scalar_tensor_tensor never does (2194ns).
  - Pool tensor_tensor runs at 0.42 sw-efficiency (4158ns) - use sparingly.
  - ACT activation is 1892ns regardless of dtype; Sin and Reciprocal live
    in different table sets (1283ns LoadActFuncSet each way), Copy/Square
    are in both.
  - 0.01/det is computed on ACT via the Reciprocal table (emitted raw: the
    bass accuracy guard is irrelevant here since accel errors are scaled
    by DT), which keeps DVE free of the f32-only reciprocal and makes the
    whole DVE chain bf16/2x.

Engine split per tile: ACT 5 ops (Sin, Square ss, Square dd, Copy g,
Reciprocal rrD), DVE 5 tt @2x + 2 stt + o23 stt [2w], Pool o45 tt [2w].
"""

import numpy as np
from contextlib import ExitStack

from concourse import bacc, tile, mybir
from concourse.alu_op_type import AluOpType
from concourse.bass_utils import run_bass_kernel_spmd

N_CORES = 8
B, P, C = 128, 16384, 6
ROWS = 128
PART = (B // N_CORES) * P // ROWS      # 2048 particles per partition per core
NI = C * PART                          # 12288 bf16 in per row
NO = 4 * PART                          # 8192 bf16 out per row
DT = 0.01
F32 = mybir.dt.float32
BF16 = mybir.dt.bfloat16


def _act_raw(nc, out, in_, func, bias=0.0, scale=1.0):
    """activation() minus the Reciprocal accuracy guard (see module doc)."""
    eng = nc.scalar
    ins = [eng.lower_ap(in_)]
    for arg in (bias, scale, 0.0):
        ins.append(mybir.ImmediateValue(dtype=mybir.dt.float32, value=arg))
    return eng.add_instruction(
        mybir.InstActivation(
            name=eng.bass.get_next_instruction_name(),
            func=func,
            ins=ins,
            outs=[eng.lower_ap(out)],
        )
    )


def _build_nc(n_t=2, io_bufs=3, tmp_bufs=2, reps=1, body="full",
              load_engine="scalar", store_engine="sync", recip_engine="act",
              o45_engine="gpsimd", o23_engine="vector"):
    # Bacc (not raw Bass): its compile() pass pipeline splits multi-sem waits
    # (walrus allows one sync wait per instruction) and allocates registers.
    nc = bacc.Bacc(
        "TRN2",
        target_bir_lowering=False,
        debug=False,
        num_devices=N_CORES,
    )
    assert PART % n_t == 0
    w = PART // n_t
    x = nc.dram_tensor("x", [ROWS, NI], BF16, kind="ExternalInput").ap()
    y = nc.dram_tensor("y", [ROWS, NO], BF16, kind="ExternalOutput").ap()

    Sin = mybir.ActivationFunctionType.Sin
    Square = mybir.ActivationFunctionType.Square
    Copy = mybir.ActivationFunctionType.Copy
    Recip = mybir.ActivationFunctionType.Reciprocal
    mult, add, sub = AluOpType.mult, AluOpType.add, AluOpType.subtract

    # activation() lowers non-Copy float biases through the const-AP table;
    # only 0.0/1.0 are pre-registered, so add the 0.5 we use for Square.
    cb = nc.alloc_sbuf_tensor("const-f32-half", [128, 1], F32)
    nc.gpsimd.memset(cb.ap(), 0.5)
    nc.const_aps.aps[(F32, 0.5)] = cb.ap()
    nc.all_engine_barrier()

    # Only SP/Activation queues can issue HWDGE DMAs. Loads go on the ACT
    # queue (they wait only on buffer-free, which resolves early, so they
    # never stall ACT work); stores sit alone on SP so their sem-wait at
    # the queue head cannot block the next iteration's load.
    engs = {"sync": nc.sync, "scalar": nc.scalar, "gpsimd": nc.gpsimd}
    load_eng = engs[load_engine]
    store_eng = engs[store_engine]
    o45e = nc.gpsimd if o45_engine == "gpsimd" else nc.vector
    o23e = nc.vector if o23_engine == "vector" else nc.gpsimd

    with tile.TileContext(nc) as tc, ExitStack() as ctx:
        io = ctx.enter_context(tc.tile_pool(name="io", bufs=io_bufs))
        tmp = ctx.enter_context(tc.tile_pool(name="tmp", bufs=tmp_bufs))

        loop = tc.For_i(0, reps, 1) if reps > 1 else None
        if loop is not None:
            ctx.enter_context(loop)

        # Two-pass emission over the n_t tiles: pass 1 does loads + the
        # trig block (+ o23, which needs no transcendentals), pass 2 the
        # Reciprocals + rational chain + stores. This keeps ACT table
        # switches at 2 per loop body instead of 2 per tile (Sin and
        # Reciprocal live in different table sets).
        tiles = []
        for j in range(n_t):
            t = io.tile([ROWS, C * w], BF16, tag="t")
            load_eng.dma_start(out=t, in_=x[:, j * C * w:(j + 1) * C * w])

            tq1 = t[:, 0 * w:1 * w]
            tq2 = t[:, 1 * w:2 * w]
            th12 = t[:, 2 * w:4 * w]
            th2 = t[:, 3 * w:4 * w]
            v12 = t[:, 4 * w:6 * w]

            if body == "dma":
                store_eng.dma_start(out=y[:, j * 4 * w:(j + 1) * 4 * w],
                                    in_=t[:, 2 * w:6 * w])
                continue

            s = tmp.tile([ROWS, w], BF16, tag="s")
            ss = tmp.tile([ROWS, w], BF16, tag="ss")
            dd = tmp.tile([ROWS, w], BF16, tag="dd")
            g = tmp.tile([ROWS, w], BF16, tag="g")

            # ---- ACT pass 1: trig chain. Sin MUST be emitted before o23
            # (o23 overwrites th12 in place; the WAR sem orders it after
            # Sin's read of the original th2).
            nc.scalar.activation(s, th2, Sin, scale=0.5)                 # sin(th2/2)
            # o23 = th12 + DT*v12, one [2w] op; depends only on the load
            o23e.scalar_tensor_tensor(th12, v12, DT, th12, mult, add)
            nc.scalar.activation(ss, s, Square)                          # ss
            nc.scalar.activation(g, ss, Copy, bias=5.0 / 6.0, scale=-1.0)
            nc.scalar.activation(dd, ss, Square, bias=0.5, scale=-1.0)   # (1/2-ss)^2
            tiles.append((t, tq1, tq2, v12, s, ss, dd, g))

        for j in range(n_t):
            if body == "dma":
                break
            t, tq1, tq2, v12, s, ss, dd, g = tiles[j]
            gm1 = tmp.tile([ROWS, w], BF16, tag="gm1")
            gm2 = tmp.tile([ROWS, w], BF16, tag="gm2")
            n1 = tmp.tile([ROWS, w], BF16, tag="n1")
            n2 = tmp.tile([ROWS, w], BF16, tag="n2")
            m12 = tmp.tile([ROWS, 2 * w], BF16, tag="m12")
            m1 = m12[:, 0:w]
            m2 = m12[:, w:2 * w]
            rr = s  # s is dead after ss; same-engine reuse is hazard-free

            if recip_engine == "act":
                # rr = 1/(-100*dd + 400/9) = 0.01/det  (DT folded in)
                _act_raw(nc, rr, dd, Recip, bias=400.0 / 9.0, scale=-100.0)
            else:
                det = tmp.tile([ROWS, w], F32, tag="det")
                rf = tmp.tile([ROWS, w], F32, tag="rf")
                nc.scalar.activation(det, dd, Copy, bias=400.0 / 9.0, scale=-100.0)
                nc.vector.reciprocal_approx_fast(rf, det)
                nc.scalar.activation(rr, rf, Copy)                       # -> bf16

            # ---- DVE: rational chain (all-bf16 keeps tt ops in 2x mode) ----
            nc.vector.tensor_tensor(gm1, g, tq1, mult)                   # g*tq1
            nc.vector.tensor_tensor(gm2, g, tq2, mult)                   # g*tq2
            nc.vector.scalar_tensor_tensor(n1, tq1, 1.0 / 3.0, gm2, mult, sub)
            nc.vector.scalar_tensor_tensor(n2, gm2, 2.0, tq2, mult, add)
            nc.vector.tensor_tensor(n2, n2, gm1, sub)                    # (2g+1)tq2 - g*tq1
            nc.vector.tensor_tensor(m1, n1, rr, mult)                    # DT*a1
            nc.vector.tensor_tensor(m2, n2, rr, mult)                    # DT*a2

            # ---- velocity updates: o45 = v12 + m12, in place ----
            o45e.tensor_tensor(v12, v12, m12, add)

            store_eng.dma_start(out=y[:, j * 4 * w:(j + 1) * 4 * w],
                                in_=t[:, 2 * w:6 * w])
    nc.finalize()
    return nc


_nc_cache = None

BEST = dict(
    n_t=2,
    io_bufs=3,
    tmp_bufs=2,
)


def _get_nc():
    global _nc_cache
    if _nc_cache is None:
        _nc_cache = _build_nc(**BEST)
    return _nc_cache


def _shard_in(prev, n_t):
    """[B, P, 6] f32 -> [8, 128, NI] bf16, per-tile channel-planar."""
    import ml_dtypes

    w = PART // n_t
    planar = (
        prev.reshape(N_CORES, ROWS, n_t, w, C)
        .transpose(0, 1, 2, 4, 3)
        .reshape(N_CORES, ROWS, NI)
    )
    return np.ascontiguousarray(planar.astype(ml_dtypes.bfloat16))


def _unshard_out(y, prev, n_t):
    """[8, 128, NO] bf16 + original f32 input -> [B, P, 6] f32."""
    w = PART // n_t
    ch = (
        y.astype(np.float32)
        .reshape(N_CORES, ROWS, n_t, 4, w)
        .transpose(0, 1, 2, 4, 3)
        .reshape(B, P, 4)
    )
    out = np.empty((B, P, C), np.float32)
    out[..., 0:2] = prev[..., 0:2]
    out[..., 2:6] = ch
    return out


def run(prev_latents, trace=False, **trace_kwargs):
    prev = np.ascontiguousarray(np.asarray(prev_latents, dtype=np.float32))
    assert prev.shape == (B, P, C), prev.shape
    n_t = BEST["n_t"]
    shards = _shard_in(prev, n_t)
    in_maps = [{"x": shards[i]} for i in range(N_CORES)]
    res = run_bass_kernel_spmd(
        _get_nc(), in_maps, list(range(N_CORES)), trace=trace, **trace_kwargs
    )
    y = np.stack([np.asarray(res.results[i]["y"]) for i in range(N_CORES)])
    return _unshard_out(y, prev, n_t), res


def kernel(**inputs):
    out, _ = run(inputs["prev_latents"])
    return out


def make_timed_runner():
    """Build a reusable jitted SPMD callable mirroring run_bass_via_pjrt's
    multi-core branch, for steady-state HW timing."""
    import jax
    from jax.sharding import Mesh, NamedSharding, PartitionSpec
    from jax.experimental.shard_map import shard_map
    from concourse import bass2jax

    nc = _get_nc()
    bass2jax.install_neuronx_cc_hook()
    partition_name = nc.partition_id_tensor.name if nc.partition_id_tensor else None

    in_names, out_names, out_avals, zero_outs = [], [], [], []
    for alloc in nc.m.functions[0].allocations:
        if not isinstance(alloc, mybir.MemoryLocationSet):
            continue
        name = alloc.memorylocations[0].name
        if alloc.kind == "ExternalInput":
            if name != partition_name:
                in_names.append(name)
        elif alloc.kind == "ExternalOutput":
            out_names.append(name)
            shape = tuple(alloc.tensor_shape)
            dtype = mybir.dt.np(alloc.dtype)
            out_avals.append(jax.core.ShapedArray(shape, dtype))
            zero_outs.append(np.zeros(shape, dtype))
    n_params, n_outs = len(in_names), len(out_avals)
    in_names.extend(out_names)
    if partition_name is not None:
        in_names.append(partition_name)
    donate = tuple(range(n_params, n_params + n_outs))

    def _body(*args):
        operands = list(args)
        if partition_name is not None:
            operands.append(bass2jax.partition_id_tensor())
        outs = bass2jax._bass_exec_p.bind(
            *operands,
            out_avals=tuple(out_avals),
            in_names=tuple(in_names),
            out_names=tuple(out_names),
            lowering_input_output_aliases=(),
            sim_require_finite=True,
            sim_require_nnan=True,
            nc=nc,
        )
        return tuple(outs)

    devices = jax.devices()[:N_CORES]
    mesh = Mesh(np.asarray(devices), ("core",))
    spec = PartitionSpec("core")
    step = jax.jit(
        shard_map(
            _body,
            mesh=mesh,
            in_specs=(spec,) * (n_params + n_outs),
            out_specs=(spec,) * n_outs,
            check_rep=False,
        ),
        donate_argnums=donate,
        keep_unused=True,
    )

    def place(arr):
        return jax.device_put(arr, NamedSharding(mesh, spec))

    concat_zeros = [
        np.zeros((N_CORES * z.shape[0], *z.shape[1:]), z.dtype) for z in zero_outs
    ]
    return step, place, concat_zeros
